# revision 37
# baseline (speedup 1.0000x reference)
"""Multi-head causal self-attention with RoPE on 8 Trainium2 NeuronCores.

Reference computation (B=2, S=2048, D=2048, H=16, DH=128):
    xs = hidden_q / sqrt(D)
    q,k,v = xs @ {Wq,Wk,Wv}.T        (reshaped to [B,H,S,DH])
    q,k <- RoPE(q,k)
    scores = q @ k.T / sqrt(DH)  (causal masked)
    p = softmax(scores); attn = p @ v
    out = (attn / sqrt(H*DH)) @ Wo.T

Sharding: 8 cores = 2 (batch) x 4 (head-groups of 4 heads).  Each core
computes its head-group's projections, attention and a partial output
projection; the host sums the 4 bf16 partials per batch in fp32.

Design (measured on TRN2; ~355 us vs 470-546 us for the fp32r baseline):
  * all matmul operands bf16 (1 col/cycle streams, weight loads pipeline
    fully behind 512-col streams; fp32r self-loading costs ~100 ns extra
    per matmul), fp32 PSUM accumulation
  * q/k/v/attn SBUF-resident for the whole kernel - no DRAM spill
  * causally trimmed streams: score/att/den matmuls only touch the live
    query range of each 128-key tile
  * scoresT layout [keys, queries]; softmax denominators via an all-ones
    [128,128] lhsT so every PSUM partition holds the row sum (reciprocal
    needs no partition broadcast); off-diagonal exp tiles are quad-summed
    on the DVE so the ones-matmul streams a quarter of the volume
  * phase B software-pipelined with a 3-tile lead so the in-order PE
    never waits on the scalar-engine exp
  * phase A RoPE on DVE in bf16; per-128-col PE transposes deferred one
    token tile so the PE never waits on the rope chain
  * PSUM->SBUF copies alternate scalar/vector; exp table, bias const and
    gpsimd ucode pre-warmed at t=0; host pre-arranges x/W for 2-4 KB DMA
    lines; y written bf16 and upcast+summed on host
"""

import math
from contextlib import ExitStack

import numpy as np
import ml_dtypes

import concourse.bass as bass
import concourse.mybir as mybir
import concourse.tile as tile
from concourse import bacc
from concourse.bass import ts
from concourse.bass_utils import run_bass_kernel_spmd
from concourse.masks import make_identity

B, S, D, H, DH = 2, 2048, 2048, 16, 128
BASE = 10000.0
G = 4              # head-groups (cores per batch)
HG = H // G        # heads per group = 4
F = HG * DH        # features per group = 512
NT = S // 128      # 16 token tiles
NQB = S // 512     # 4 query blocks
F32 = mybir.dt.float32
BF16 = mybir.dt.bfloat16

_cache = {}


def _rope_tables():
    inv_freq = 1.0 / (BASE ** (np.arange(0, DH, 2, dtype=np.float64) / DH))
    t = np.arange(S, dtype=np.float64)
    freqs = np.outer(t, inv_freq)                       # [S, 64]
    return (np.cos(freqs), np.sin(freqs))


def _mask_tiles():
    # mask[o][j, q] = 1 if q >= j + 128*o else 0  (diagonal-band tiles)
    o = np.arange(4)[:, None, None]
    j = np.arange(128)[None, :, None]
    q = np.arange(512)[None, None, :]
    return (q >= j + 128 * o).astype(np.float64)        # [4, 128, 512]


def _build(reps=1):
    key = ("nc", reps)
    if key in _cache:
        return _cache[key]
    nc = bacc.Bacc("TRN2", target_bir_lowering=False, debug=False, num_devices=8)

    # host-prearranged partition-major layouts for wide DMA lines
    xT = nc.dram_tensor("xT", [128, NT, NT * 128], BF16, kind="ExternalInput")
    wqT = nc.dram_tensor("wqT", [128, NT * F], BF16, kind="ExternalInput")
    wkT = nc.dram_tensor("wkT", [128, NT * F], BF16, kind="ExternalInput")
    wvT = nc.dram_tensor("wvT", [128, NT * F], BF16, kind="ExternalInput")
    woT = nc.dram_tensor("woT", [F, D], BF16, kind="ExternalInput")
    cos_d = nc.dram_tensor("cos", [S, 64], BF16, kind="ExternalInput")
    sin_d = nc.dram_tensor("sin", [S, 64], BF16, kind="ExternalInput")
    msk_d = nc.dram_tensor("masks", [4, 128, 512], BF16, kind="ExternalInput")
    y = nc.dram_tensor("y", [S, D], BF16, kind="ExternalOutput")



    with tile.TileContext(nc) as tc, ExitStack() as ctx:
        const = ctx.enter_context(tc.tile_pool(name="const", bufs=1))
        persist = ctx.enter_context(tc.tile_pool(name="persist", bufs=1))

        ones_f = const.tile([128, 1], F32, tag="ones_f")
        nc.gpsimd.memset(ones_f[:], 1.0)
        ones = const.tile([128, 128], BF16, tag="ones")
        nc.vector.tensor_copy(ones[:], ones_f[:].broadcast_to((128, 128)))
        # pre-warm the scalar-engine exp table + bias const off the critical path
        warm = const.tile([128, 8], F32, tag="warm")
        nc.gpsimd.memset(warm[:], 0.0)
        nc.scalar.activation(warm[:], warm[:], mybir.ActivationFunctionType.Exp,
                             scale=1.0)
        gwarm = const.tile([128, 8], F32, tag="gwarm")
        nc.gpsimd.partition_broadcast(gwarm[:], warm[0:1, :])
        ident_f = const.tile([128, 128], F32, tag="ident_f")
        make_identity(nc, ident_f[:])
        ident = const.tile([128, 128], BF16, tag="ident")
        nc.vector.tensor_copy(ident[:], ident_f[:])

        for _rep in range(reps):
            # persistent SBUF tensors (bf16): q/k transposed, v natural,
            # normalized attn, output weights, masks, rope tables
            qh = persist.tile([128, HG, S], BF16, tag="qh", name="qh")
            kh = persist.tile([128, HG, S], BF16, tag="kh", name="kh")
            vh = persist.tile([128, NT, F], BF16, tag="vh", name="vh")
            attn_sb = persist.tile([128, HG, S], BF16, tag="attn_sb")
            wo_sb = persist.tile([128, G, D], BF16, tag="wo")
            msk_sb = persist.tile([128, 4, 512], BF16, tag="msk")
            cos_sb = persist.tile([128, NT, 64], BF16, tag="cos")
            sin_sb = persist.tile([128, NT, 64], BF16, tag="sin")

            # ---------- Phase A: projections + RoPE + transpose --------
            with ExitStack() as actx:
                wpool = actx.enter_context(tc.tile_pool(name="wpool", bufs=1))
                xpool = actx.enter_context(tc.tile_pool(name="xpool", bufs=3))
                rpool = actx.enter_context(tc.tile_pool(name="rot", bufs=2))
                tmp_pool = actx.enter_context(tc.tile_pool(name="tmp", bufs=2))
                ps512 = actx.enter_context(
                    tc.tile_pool(name="ps512", bufs=6, space="PSUM"))
                ps_t = actx.enter_context(
                    tc.tile_pool(name="ps_t", bufs=2, space="PSUM"))

                wq_sb = wpool.tile([128, NT, F], BF16, tag="wq")
                wk_sb = wpool.tile([128, NT, F], BF16, tag="wk")
                wv_sb = wpool.tile([128, NT, F], BF16, tag="wv")
                wqT_r = wqT.ap().rearrange("p (kp c) -> p kp c", kp=NT // 2)
                wkT_r = wkT.ap().rearrange("p (kp c) -> p kp c", kp=NT // 2)
                wvT_r = wvT.ap().rearrange("p (kp c) -> p kp c", kp=NT // 2)
                # weight/x DMA stream ordered so the PE can start at kt=0;
                # kt-pair transfers give 2 KB DMA lines
                xq_tiles = {}
                for tb in range(3):
                    xq_tiles[tb] = xpool.tile([128, NT, 128], BF16, tag="xq",
                                              name=f"xq{tb}")
                # opening transfers split by partition halves across queues
                for pp in range(2):
                    pr = slice(64 * pp, 64 * (pp + 1))
                    nc.sync.dma_start(
                        xq_tiles[0][pr, 0:4, :],
                        xT.ap()[pr, 0, 0:512]
                        .rearrange("p (kt t) -> p kt t", kt=4))
                for (wsb, wr) in ((wq_sb, wqT_r), (wk_sb, wkT_r), (wv_sb, wvT_r)):
                    for pp in range(2):
                        pr = slice(64 * pp, 64 * (pp + 1))
                        nc.sync.dma_start(
                            wsb[pr, 0:2, :].rearrange("p a b -> p (a b)"),
                            wr[pr, 0, :])
                for ck in range(1, 4):
                    nc.sync.dma_start(
                        xq_tiles[0][:, 4 * ck:4 * ck + 4, :],
                        xT.ap()[:, 0, 512 * ck:512 * (ck + 1)]
                        .rearrange("p (kt t) -> p kt t", kt=4))
                for kp in range(1, NT // 2):
                    nc.sync.dma_start(
                        wq_sb[:, 2 * kp:2 * kp + 2, :]
                        .rearrange("p a b -> p (a b)"), wqT_r[:, kp, :])
                    nc.sync.dma_start(
                        wk_sb[:, 2 * kp:2 * kp + 2, :]
                        .rearrange("p a b -> p (a b)"), wkT_r[:, kp, :])
                    nc.sync.dma_start(
                        wv_sb[:, 2 * kp:2 * kp + 2, :]
                        .rearrange("p a b -> p (a b)"), wvT_r[:, kp, :])
                    if kp in (1, 3):
                        tb = 1 if kp == 1 else 2
                        nc.sync.dma_start(xq_tiles[tb][:], xT.ap()[:, tb, :]
                                          .rearrange("p (kt t) -> p kt t", kt=NT))
                # later-phase constants after the critical weight stream
                nc.sync.dma_start(cos_sb[:],
                                  cos_d.ap().rearrange("(t p) c -> p t c", p=128))
                nc.sync.dma_start(sin_sb[:],
                                  sin_d.ap().rearrange("(t p) c -> p t c", p=128))
                nc.sync.dma_start(msk_sb[:],
                                  msk_d.ap().rearrange("o p q -> p o q"))
                nc.sync.dma_start(wo_sb[:],
                                  woT.ap().rearrange("(ft p) d -> p ft d", p=128))

                def emit_transposes(tb, rots):
                    # PE transposes for tile tb (rot tiles already computed on
                    # DVE); deferred one tb so the PE never waits on the rope.
                    for (rot, dstT) in rots:
                        ptr = ps_t.tile([128, HG, 128], BF16, tag="ptr")
                        for hb in range(HG):
                            nc.tensor.transpose(ptr[:, hb, :],
                                                rot[:, ts(hb, 128)], ident[:])
                        if tb % 2 == 0:
                            nc.scalar.copy(dstT[:, :, ts(tb, 128)], ptr[:])
                        else:
                            nc.vector.tensor_copy(dstT[:, :, ts(tb, 128)], ptr[:])

                pending = None
                for tb in range(NT):
                    if tb in xq_tiles:
                        xq = xq_tiles[tb]
                    else:
                        xq = xpool.tile([128, NT, 128], BF16, tag="xq")
                        nc.sync.dma_start(xq[:], xT.ap()[:, tb, :]
                                          .rearrange("p (kt t) -> p kt t", kt=NT))
                    pq = ps512.tile([128, 512], F32, tag="ps512")
                    pk = ps512.tile([128, 512], F32, tag="ps512")
                    pv = ps512.tile([128, 512], F32, tag="ps512")
                    for kt in range(NT):
                        f = dict(start=(kt == 0), stop=(kt == NT - 1))
                        nc.tensor.matmul(pq[:], xq[:, kt, :], wq_sb[:, kt, :], **f)
                        nc.tensor.matmul(pk[:], xq[:, kt, :], wk_sb[:, kt, :], **f)
                        nc.tensor.matmul(pv[:], xq[:, kt, :], wv_sb[:, kt, :], **f)
                    if tb % 2 == 0:
                        nc.vector.tensor_copy(vh[:, tb, :], pv[:])
                    else:
                        nc.scalar.copy(vh[:, tb, :], pv[:])

                    # RoPE: copy PSUM->bf16 once, then 4 wide DVE ops at
                    # 16-bit rate; transpose per 128-col block on the PE.
                    cos_b = cos_sb[:, tb, :].unsqueeze(1).unsqueeze(1) \
                        .broadcast_to((128, HG, 2, 64))
                    sin_b = sin_sb[:, tb, :].unsqueeze(1).broadcast_to((128, HG, 64))
                    rots = []
                    for (ps, dstT, rtag) in ((pq, qh, "q"), (pk, kh, "k")):
                        qsb = rpool.tile([128, 512], BF16, tag=f"sb{rtag}")
                        if tb % 2 == 0:
                            nc.scalar.copy(qsb[:], ps[:])
                        else:
                            nc.vector.tensor_copy(qsb[:], ps[:])
                        qsb_r = qsb[:].rearrange("p (hb half j) -> p hb half j",
                                                 hb=HG, half=2, j=64)
                        rot = rpool.tile([128, 512], BF16, tag=f"rot{rtag}")
                        rot_r = rot[:].rearrange("p (hb half j) -> p hb half j",
                                                 hb=HG, half=2, j=64)
                        tmp = tmp_pool.tile([128, HG, 2, 64], BF16, tag="tmp")
                        # tmp_lo = -q_hi * sin ; tmp_hi = +q_lo * sin
                        nc.vector.scalar_tensor_tensor(
                            tmp[:, :, 0, :], qsb_r[:, :, 1, :], -1.0, sin_b,
                            op0=mybir.AluOpType.mult, op1=mybir.AluOpType.mult)
                        nc.vector.tensor_mul(tmp[:, :, 1, :], qsb_r[:, :, 0, :], sin_b)
                        # rot = q * cos + tmp
                        nc.vector.tensor_mul(rot_r[:], qsb_r[:], cos_b)
                        nc.vector.tensor_add(rot[:], rot[:],
                                             tmp[:].rearrange("p a b c -> p (a b c)"))
                        rots.append((rot, dstT))
                    if pending is not None:
                        emit_transposes(pending[0], pending[1])
                    pending = (tb, rots)
                emit_transposes(pending[0], pending[1])

            # ---------- Phase B+C: attention + output projection --------
            with ExitStack() as bctx:
                pt_pool = bctx.enter_context(tc.tile_pool(name="pt", bufs=4))
                nrm = bctx.enter_context(tc.tile_pool(name="nrm", bufs=4))
                ystage = bctx.enter_context(tc.tile_pool(name="ystage", bufs=6))
                ps_s = bctx.enter_context(
                    tc.tile_pool(name="ps_s", bufs=3, space="PSUM"))
                ps_att = bctx.enter_context(
                    tc.tile_pool(name="ps_att", bufs=2, space="PSUM"))
                ps_y = bctx.enter_context(
                    tc.tile_pool(name="ps_y", bufs=3, space="PSUM"))

                for qb in range(NQB):
                    nkt = 4 * qb + 4
                    for h in range(HG):
                        p_att = ps_att.tile([128, 512], F32, tag="p_att")
                        den = ps_y.tile([128, 512], F32, tag="ps_y", name="den")
                        st = {"started": False, "prev": None, "n": 0}

                        def consume(kt, pt, qo, diag, st=st, qb=qb, h=h,
                                    nkt=nkt, p_att=p_att, den=den):
                            nc.tensor.matmul(p_att[:, qo:],
                                             vh[:, kt, ts(h, 128)],
                                             pt[:, qo:],
                                             start=(kt == 0), stop=(kt == nkt - 1))
                            # denominator: off-diagonal tiles quad-summed on
                            # the DVE so the ones-matmul streams a quarter
                            if not diag:
                                if st["prev"] is None:
                                    st["prev"] = pt
                                    st["n"] = 1
                                elif st["n"] == 1:
                                    pts = pt_pool.tile([128, 512], BF16,
                                                       tag="ptsum", bufs=3,
                                                       name="pts")
                                    nc.vector.tensor_add(pts[:], st["prev"][:],
                                                         pt[:])
                                    st["prev"] = pts
                                    st["n"] = 2
                                else:
                                    pts = st["prev"]
                                    nc.vector.tensor_add(pts[:], pts[:], pt[:])
                                    st["n"] += 1
                                    if st["n"] == 4:
                                        nc.tensor.matmul(
                                            den[:], ones[:], pts[:],
                                            start=not st["started"], stop=False)
                                        st["started"] = True
                                        st["prev"] = None
                                        st["n"] = 0
                            else:
                                nc.tensor.matmul(den[:, qo:], ones[:], pt[:, qo:],
                                                 start=not st["started"],
                                                 stop=(kt == nkt - 1))
                                st["started"] = True

                        # software pipeline: emit p_s two tiles ahead of the
                        # consumers so the in-order PE never waits on the exp
                        pend = []
                        for kt in range(nkt):
                            qo = max(0, 128 * (kt - 4 * qb))
                            p_s = ps_s.tile([128, 512], F32, tag="p_s")
                            nc.tensor.matmul(p_s[:, qo:],
                                             kh[:, h, ts(kt, 128)],
                                             qh[:, h, 512 * qb + qo:512 * (qb + 1)],
                                             start=True, stop=True)
                            pt = pt_pool.tile([128, 512], BF16, tag="pt", bufs=6)
                            nc.scalar.activation(
                                pt[:, qo:], p_s[:, qo:],
                                mybir.ActivationFunctionType.Exp,
                                scale=1.0 / math.sqrt(DH))
                            diag = kt >= 4 * qb
                            if diag:
                                nc.vector.tensor_mul(
                                    pt[:, qo:], pt[:, qo:],
                                    msk_sb[:, kt - 4 * qb, qo:])
                            pend.append((kt, pt, qo, diag))
                            if len(pend) > 3:
                                consume(*pend.pop(0))
                        for it in pend:
                            consume(*it)
                        # den rows are identical (all-ones lhsT), so the
                        # reciprocal is computed directly on all partitions
                        rb = nrm.tile([128, 512], F32, tag="rb")
                        nc.vector.reciprocal_approx_fast(rb[:], den[:, :])
                        nc.vector.tensor_mul(attn_sb[:, h, ts(qb, 512)],
                                             p_att[:], rb[:])
                    # output projection for this query block
                    for qt in range(4 * qb, 4 * qb + 4):
                        for ddb in range(NQB):
                            py = ps_y.tile([128, 512], F32, tag="ps_y")
                            for ft in range(G):
                                nc.tensor.matmul(py[:], attn_sb[:, ft, ts(qt, 128)],
                                                 wo_sb[:, ft, ts(ddb, 512)],
                                                 start=(ft == 0), stop=(ft == G - 1))
                            y_sb = ystage.tile([128, 512], BF16, tag="ysb")
                            nc.vector.tensor_copy(y_sb[:], py[:])
                            if ddb % 2 == 0:
                                nc.sync.dma_start(
                                    y.ap()[ts(qt, 128), ts(ddb, 512)], y_sb[:])
                            else:
                                nc.gpsimd.dma_start(
                                    y.ap()[ts(qt, 128), ts(ddb, 512)], y_sb[:])

    nc.compile()
    _cache[key] = nc
    return nc


def _in_maps(hidden_q, Wq, Wk, Wv, Wo):
    bf = ml_dtypes.bfloat16

    def warr(wT):
        # [D, F] -> [128, NT*F] with arr[p, kt*F+f] = wT[kt*128+p, f]
        return np.ascontiguousarray(
            wT.reshape(NT, 128, F).transpose(1, 0, 2).reshape(128, NT * F)
        ).astype(bf)

    xs = (hidden_q.astype(np.float64) / math.sqrt(D))
    xTl = []
    for b in range(B):
        xT_b = xs[b].T                                       # [D, S]
        # [128, tb, kt*128] with arr[p, tb, kt*128+t] = xT[kt*128+p, tb*128+t]
        xTl.append(np.ascontiguousarray(
            xT_b.reshape(NT, 128, NT, 128).transpose(1, 2, 0, 3)
            .reshape(128, NT, NT * 128)).astype(bf))
    cos_t, sin_t = _rope_tables()
    cos_t, sin_t = cos_t.astype(bf), sin_t.astype(bf)
    masks = _mask_tiles().astype(bf)
    wo_s = Wo.astype(np.float64) / math.sqrt(H * DH)
    in_maps = []
    for c in range(8):
        b, g = c // G, c % G
        rows = slice(F * g, F * (g + 1))
        in_maps.append({
            "xT": xTl[b],
            "wqT": warr(Wq[rows, :].T),
            "wkT": warr(Wk[rows, :].T),
            "wvT": warr(Wv[rows, :].T),
            "woT": np.ascontiguousarray(wo_s[:, rows].T).astype(bf),
            "cos": cos_t, "sin": sin_t, "masks": masks,
        })
    return in_maps


def kernel(hidden_q, attention_mask, position_bias, Wq, Wk, Wv, Wo):
    hidden_q = np.asarray(hidden_q)
    Wq, Wk, Wv, Wo = (np.asarray(w) for w in (Wq, Wk, Wv, Wo))
    assert hidden_q.shape == (B, S, D)
    in_maps = _in_maps(hidden_q, Wq, Wk, Wv, Wo)
    nc = _build()
    res = run_bass_kernel_spmd(nc, in_maps, core_ids=list(range(8)))
    _cache["last_results"] = res
    out = np.zeros((B, S, D), np.float32)
    for c in range(8):
        out[c // G] += res.results[c]["y"].astype(np.float32)
    return out


# revision 48
# speedup vs baseline: 1.0167x; 1.0167x over previous
"""Multi-head causal self-attention with RoPE on 8 Trainium2 NeuronCores.

Reference computation (B=2, S=2048, D=2048, H=16, DH=128):
    xs = hidden_q / sqrt(D)
    q,k,v = xs @ {Wq,Wk,Wv}.T        (reshaped to [B,H,S,DH])
    q,k <- RoPE(q,k)
    scores = q @ k.T / sqrt(DH)  (causal masked)
    p = softmax(scores); attn = p @ v
    out = (attn / sqrt(H*DH)) @ Wo.T

Sharding: 8 cores = 2 (batch) x 4 (head-groups of 4 heads).  Each core
computes its head-group's projections, attention and a partial output
projection; the host sums the 4 bf16 partials per batch in fp32.

Design (measured on TRN2; ~355 us vs 470-546 us for the fp32r baseline):
  * all matmul operands bf16 (1 col/cycle streams, weight loads pipeline
    fully behind 512-col streams; fp32r self-loading costs ~100 ns extra
    per matmul), fp32 PSUM accumulation
  * q/k/v/attn SBUF-resident for the whole kernel - no DRAM spill
  * causally trimmed streams: score/att/den matmuls only touch the live
    query range of each 128-key tile
  * scoresT layout [keys, queries]; softmax denominators via an all-ones
    [128,128] lhsT so every PSUM partition holds the row sum (reciprocal
    needs no partition broadcast); off-diagonal exp tiles are quad-summed
    on the DVE so the ones-matmul streams a quarter of the volume
  * phase B software-pipelined with a 3-tile lead so the in-order PE
    never waits on the scalar-engine exp
  * phase A RoPE on DVE in bf16; per-128-col PE transposes deferred one
    token tile so the PE never waits on the rope chain
  * PSUM->SBUF copies alternate scalar/vector; exp table, bias const and
    gpsimd ucode pre-warmed at t=0; host pre-arranges x/W for 2-4 KB DMA
    lines; y written bf16 and upcast+summed on host
"""

import math
from contextlib import ExitStack

import numpy as np
import ml_dtypes

import concourse.bass as bass
import concourse.mybir as mybir
import concourse.tile as tile
from concourse import bacc
from concourse.bass import ts
from concourse.bass_utils import run_bass_kernel_spmd
from concourse.masks import make_identity

B, S, D, H, DH = 2, 2048, 2048, 16, 128
BASE = 10000.0
G = 4              # head-groups (cores per batch)
HG = H // G        # heads per group = 4
F = HG * DH        # features per group = 512
NT = S // 128      # 16 token tiles
NQB = S // 512     # 4 query blocks
F32 = mybir.dt.float32
BF16 = mybir.dt.bfloat16

_cache = {}


def _rope_tables():
    inv_freq = 1.0 / (BASE ** (np.arange(0, DH, 2, dtype=np.float64) / DH))
    t = np.arange(S, dtype=np.float64)
    freqs = np.outer(t, inv_freq)                       # [S, 64]
    return (np.cos(freqs), np.sin(freqs))


def _mask_tiles():
    # mask[o][j, q] = 1 if q >= j + 128*o else 0  (diagonal-band tiles)
    o = np.arange(4)[:, None, None]
    j = np.arange(128)[None, :, None]
    q = np.arange(512)[None, None, :]
    return (q >= j + 128 * o).astype(np.float64)        # [4, 128, 512]


def _build(reps=1):
    key = ("nc", reps)
    if key in _cache:
        return _cache[key]
    nc = bacc.Bacc("TRN2", target_bir_lowering=False, debug=False, num_devices=8)

    # host-prearranged partition-major layouts for wide DMA lines
    xT = nc.dram_tensor("xT", [128, NT, NT * 128], BF16, kind="ExternalInput")
    wqT = nc.dram_tensor("wqT", [128, NT * F], BF16, kind="ExternalInput")
    wkT = nc.dram_tensor("wkT", [128, NT * F], BF16, kind="ExternalInput")
    wvT = nc.dram_tensor("wvT", [128, NT * F], BF16, kind="ExternalInput")
    woT = nc.dram_tensor("woT", [F, D], BF16, kind="ExternalInput")
    cos_d = nc.dram_tensor("cos", [S, 64], BF16, kind="ExternalInput")
    sin_d = nc.dram_tensor("sin", [S, 64], BF16, kind="ExternalInput")
    msk_d = nc.dram_tensor("masks", [4, 128, 512], BF16, kind="ExternalInput")
    y = nc.dram_tensor("y", [S, D], BF16, kind="ExternalOutput")



    with tile.TileContext(nc) as tc, ExitStack() as ctx:
        const = ctx.enter_context(tc.tile_pool(name="const", bufs=1))
        persist = ctx.enter_context(tc.tile_pool(name="persist", bufs=1))

        ones_f = const.tile([128, 1], F32, tag="ones_f")
        nc.gpsimd.memset(ones_f[:], 1.0)
        ones = const.tile([128, 128], BF16, tag="ones")
        nc.vector.tensor_copy(ones[:], ones_f[:].broadcast_to((128, 128)))
        # pre-warm the scalar-engine exp table + bias const off the critical path
        warm = const.tile([128, 8], F32, tag="warm")
        nc.gpsimd.memset(warm[:], 0.0)
        nc.scalar.activation(warm[:], warm[:], mybir.ActivationFunctionType.Exp,
                             scale=1.0)
        gwarm = const.tile([128, 8], F32, tag="gwarm")
        nc.gpsimd.partition_broadcast(gwarm[:], warm[0:1, :])
        ident_f = const.tile([128, 128], F32, tag="ident_f")
        make_identity(nc, ident_f[:])
        ident = const.tile([128, 128], BF16, tag="ident")
        nc.vector.tensor_copy(ident[:], ident_f[:])

        for _rep in range(reps):
            # persistent SBUF tensors (bf16): q/k transposed, v natural,
            # normalized attn, output weights, masks, rope tables
            qh = persist.tile([128, HG, S], BF16, tag="qh", name="qh")
            kh = persist.tile([128, HG, S], BF16, tag="kh", name="kh")
            vh = persist.tile([128, NT, F], BF16, tag="vh", name="vh")
            attn_sb = persist.tile([128, HG, S], BF16, tag="attn_sb")
            wo_sb = persist.tile([128, G, D], BF16, tag="wo")
            msk_sb = persist.tile([128, 4, 512], BF16, tag="msk")
            cos_sb = persist.tile([128, NT, 64], BF16, tag="cos")
            sin_sb = persist.tile([128, NT, 64], BF16, tag="sin")

            # ---------- Phase A: projections + RoPE + transpose --------
            with ExitStack() as actx:
                wpool = actx.enter_context(tc.tile_pool(name="wpool", bufs=1))
                xpool = actx.enter_context(tc.tile_pool(name="xpool", bufs=3))
                rpool = actx.enter_context(tc.tile_pool(name="rot", bufs=2))
                tmp_pool = actx.enter_context(tc.tile_pool(name="tmp", bufs=2))
                ps512 = actx.enter_context(
                    tc.tile_pool(name="ps512", bufs=6, space="PSUM"))
                ps_t = actx.enter_context(
                    tc.tile_pool(name="ps_t", bufs=2, space="PSUM"))

                wq_sb = wpool.tile([128, NT, F], BF16, tag="wq")
                wk_sb = wpool.tile([128, NT, F], BF16, tag="wk")
                wv_sb = wpool.tile([128, NT, F], BF16, tag="wv")
                wqT_r = wqT.ap().rearrange("p (kp c) -> p kp c", kp=NT // 2)
                wkT_r = wkT.ap().rearrange("p (kp c) -> p kp c", kp=NT // 2)
                wvT_r = wvT.ap().rearrange("p (kp c) -> p kp c", kp=NT // 2)
                # weight/x DMA stream ordered so the PE can start at kt=0;
                # kt-pair transfers give 2 KB DMA lines
                xq_tiles = {}
                for tb in range(3):
                    xq_tiles[tb] = xpool.tile([128, NT, 128], BF16, tag="xq",
                                              name=f"xq{tb}")
                # opening transfers split by partition halves across queues
                for pp in range(2):
                    pr = slice(64 * pp, 64 * (pp + 1))
                    nc.sync.dma_start(
                        xq_tiles[0][pr, 0:4, :],
                        xT.ap()[pr, 0, 0:512]
                        .rearrange("p (kt t) -> p kt t", kt=4))
                for (wsb, wr) in ((wq_sb, wqT_r), (wk_sb, wkT_r), (wv_sb, wvT_r)):
                    for pp in range(2):
                        pr = slice(64 * pp, 64 * (pp + 1))
                        nc.sync.dma_start(
                            wsb[pr, 0:2, :].rearrange("p a b -> p (a b)"),
                            wr[pr, 0, :])
                for ck in range(1, 4):
                    nc.sync.dma_start(
                        xq_tiles[0][:, 4 * ck:4 * ck + 4, :],
                        xT.ap()[:, 0, 512 * ck:512 * (ck + 1)]
                        .rearrange("p (kt t) -> p kt t", kt=4))
                for kp in range(1, NT // 2):
                    nc.sync.dma_start(
                        wq_sb[:, 2 * kp:2 * kp + 2, :]
                        .rearrange("p a b -> p (a b)"), wqT_r[:, kp, :])
                    nc.sync.dma_start(
                        wk_sb[:, 2 * kp:2 * kp + 2, :]
                        .rearrange("p a b -> p (a b)"), wkT_r[:, kp, :])
                    nc.sync.dma_start(
                        wv_sb[:, 2 * kp:2 * kp + 2, :]
                        .rearrange("p a b -> p (a b)"), wvT_r[:, kp, :])
                    if kp in (1, 3):
                        tb = 1 if kp == 1 else 2
                        nc.sync.dma_start(xq_tiles[tb][:], xT.ap()[:, tb, :]
                                          .rearrange("p (kt t) -> p kt t", kt=NT))
                # later-phase constants after the critical weight stream
                nc.sync.dma_start(cos_sb[:],
                                  cos_d.ap().rearrange("(t p) c -> p t c", p=128))
                nc.sync.dma_start(sin_sb[:],
                                  sin_d.ap().rearrange("(t p) c -> p t c", p=128))
                nc.sync.dma_start(msk_sb[:],
                                  msk_d.ap().rearrange("o p q -> p o q"))
                nc.sync.dma_start(wo_sb[:],
                                  woT.ap().rearrange("(ft p) d -> p ft d", p=128))

                def emit_transposes(tb, rots):
                    # PE transposes for tile tb (rot tiles already computed on
                    # DVE); deferred one tb so the PE never waits on the rope.
                    for (rot, dstT) in rots:
                        ptr = ps_t.tile([128, HG, 128], BF16, tag="ptr")
                        for hb in range(HG):
                            nc.tensor.transpose(ptr[:, hb, :],
                                                rot[:, ts(hb, 128)], ident[:])
                        if tb % 2 == 0:
                            nc.scalar.copy(dstT[:, :, ts(tb, 128)], ptr[:])
                        else:
                            nc.vector.tensor_copy(dstT[:, :, ts(tb, 128)], ptr[:])

                pending = None
                for tb in range(NT):
                    if tb in xq_tiles:
                        xq = xq_tiles[tb]
                    else:
                        xq = xpool.tile([128, NT, 128], BF16, tag="xq")
                        nc.sync.dma_start(xq[:], xT.ap()[:, tb, :]
                                          .rearrange("p (kt t) -> p kt t", kt=NT))
                    pq = ps512.tile([128, 512], F32, tag="ps512")
                    pk = ps512.tile([128, 512], F32, tag="ps512")
                    pv = ps512.tile([128, 512], F32, tag="ps512")
                    for kt in range(NT):
                        f = dict(start=(kt == 0), stop=(kt == NT - 1))
                        nc.tensor.matmul(pq[:], xq[:, kt, :], wq_sb[:, kt, :], **f)
                        nc.tensor.matmul(pk[:], xq[:, kt, :], wk_sb[:, kt, :], **f)
                        nc.tensor.matmul(pv[:], xq[:, kt, :], wv_sb[:, kt, :], **f)
                    if tb % 2 == 0:
                        nc.vector.tensor_copy(vh[:, tb, :], pv[:])
                    else:
                        nc.scalar.copy(vh[:, tb, :], pv[:])

                    # RoPE: copy PSUM->bf16 once, then 4 wide DVE ops at
                    # 16-bit rate; transpose per 128-col block on the PE.
                    cos_b = cos_sb[:, tb, :].unsqueeze(1).unsqueeze(1) \
                        .broadcast_to((128, HG, 2, 64))
                    sin_b = sin_sb[:, tb, :].unsqueeze(1).broadcast_to((128, HG, 64))
                    rots = []
                    for (ps, dstT, rtag) in ((pq, qh, "q"), (pk, kh, "k")):
                        qsb = rpool.tile([128, 512], BF16, tag=f"sb{rtag}")
                        if tb % 2 == 0:
                            nc.scalar.copy(qsb[:], ps[:])
                        else:
                            nc.vector.tensor_copy(qsb[:], ps[:])
                        qsb_r = qsb[:].rearrange("p (hb half j) -> p hb half j",
                                                 hb=HG, half=2, j=64)
                        rot = rpool.tile([128, 512], BF16, tag=f"rot{rtag}")
                        rot_r = rot[:].rearrange("p (hb half j) -> p hb half j",
                                                 hb=HG, half=2, j=64)
                        tmp = tmp_pool.tile([128, HG, 2, 64], BF16, tag="tmp")
                        # tmp_lo = -q_hi * sin ; tmp_hi = +q_lo * sin
                        nc.vector.scalar_tensor_tensor(
                            tmp[:, :, 0, :], qsb_r[:, :, 1, :], -1.0, sin_b,
                            op0=mybir.AluOpType.mult, op1=mybir.AluOpType.mult)
                        nc.vector.tensor_mul(tmp[:, :, 1, :], qsb_r[:, :, 0, :], sin_b)
                        # rot = q * cos + tmp
                        nc.vector.tensor_mul(rot_r[:], qsb_r[:], cos_b)
                        nc.vector.tensor_add(rot[:], rot[:],
                                             tmp[:].rearrange("p a b c -> p (a b c)"))
                        rots.append((rot, dstT))
                    if pending is not None:
                        emit_transposes(pending[0], pending[1])
                    pending = (tb, rots)
                emit_transposes(pending[0], pending[1])

            # ---------- Phase B+C: attention + output projection --------
            with ExitStack() as bctx:
                pt_pool = bctx.enter_context(tc.tile_pool(name="pt", bufs=4))
                nrm = bctx.enter_context(tc.tile_pool(name="nrm", bufs=4))
                ystage = bctx.enter_context(tc.tile_pool(name="ystage", bufs=6))
                ps_s = bctx.enter_context(
                    tc.tile_pool(name="ps_s", bufs=3, space="PSUM"))
                ps_att = bctx.enter_context(
                    tc.tile_pool(name="ps_att", bufs=2, space="PSUM"))
                ps_y = bctx.enter_context(
                    tc.tile_pool(name="ps_y", bufs=3, space="PSUM"))

                def emit_ps(qb, h, kt):
                    # score matmul + exp (+ mask) for one 128-key tile
                    qo = max(0, 128 * (kt - 4 * qb))
                    p_s = ps_s.tile([128, 512], F32, tag="p_s", name="p_s")
                    nc.tensor.matmul(p_s[:, qo:],
                                     kh[:, h, ts(kt, 128)],
                                     qh[:, h, 512 * qb + qo:512 * (qb + 1)],
                                     start=True, stop=True)
                    pt = pt_pool.tile([128, 512], BF16, tag="pt", bufs=6,
                                      name="pt")
                    nc.scalar.activation(
                        pt[:, qo:], p_s[:, qo:],
                        mybir.ActivationFunctionType.Exp,
                        scale=1.0 / math.sqrt(DH))
                    diag = kt >= 4 * qb
                    if diag:
                        nc.vector.tensor_mul(
                            pt[:, qo:], pt[:, qo:],
                            msk_sb[:, kt - 4 * qb, qo:])
                    return (kt, pt, qo, diag)

                peeled = {}
                for qb in range(NQB):
                    nkt = 4 * qb + 4
                    for h in range(HG):
                        p_att = ps_att.tile([128, 512], F32, tag="p_att")
                        den = ps_y.tile([128, 512], F32, tag="ps_y", name="den")
                        st = {"started": False, "prev": None, "n": 0}

                        def consume(kt, pt, qo, diag, st=st, qb=qb, h=h,
                                    nkt=nkt, p_att=p_att, den=den):
                            nc.tensor.matmul(p_att[:, qo:],
                                             vh[:, kt, ts(h, 128)],
                                             pt[:, qo:],
                                             start=(kt == 0), stop=(kt == nkt - 1))
                            # denominator: off-diagonal tiles quad-summed on
                            # the DVE so the ones-matmul streams a quarter
                            if not diag:
                                if st["prev"] is None:
                                    st["prev"] = pt
                                    st["n"] = 1
                                elif st["n"] == 1:
                                    pts = pt_pool.tile([128, 512], BF16,
                                                       tag="ptsum", bufs=3,
                                                       name="pts")
                                    nc.vector.tensor_add(pts[:], st["prev"][:],
                                                         pt[:])
                                    st["prev"] = pts
                                    st["n"] = 2
                                else:
                                    pts = st["prev"]
                                    nc.vector.tensor_add(pts[:], pts[:], pt[:])
                                    st["n"] += 1
                                    if st["n"] == 4:
                                        nc.tensor.matmul(
                                            den[:], ones[:], pts[:],
                                            start=not st["started"], stop=False)
                                        st["started"] = True
                                        st["prev"] = None
                                        st["n"] = 0
                            else:
                                nc.tensor.matmul(den[:, qo:], ones[:], pt[:, qo:],
                                                 start=not st["started"],
                                                 stop=(kt == nkt - 1))
                                st["started"] = True

                        # software pipeline: emit p_s three tiles ahead of the
                        # consumers so the in-order PE never waits on the exp
                        pend = list(peeled.pop((qb, h), []))
                        for kt in range(len(pend), nkt):
                            pend.append(emit_ps(qb, h, kt))
                            if len(pend) > 3:
                                consume(*pend.pop(0))
                        # peel the next head's first scores before draining
                        # this head's tail: its exps run during the tail's
                        # p_att/den matmuls
                        if h + 1 < HG:
                            peeled[(qb, h + 1)] = [emit_ps(qb, h + 1, kt)
                                                   for kt in range(3)]
                        for it in pend:
                            consume(*it)
                        # den rows are identical (all-ones lhsT), so the
                        # reciprocal is computed directly on all partitions
                        rb = nrm.tile([128, 512], F32, tag="rb")
                        nc.vector.reciprocal_approx_fast(rb[:], den[:, :])
                        nc.vector.tensor_mul(attn_sb[:, h, ts(qb, 512)],
                                             p_att[:], rb[:])
                    # peel the next query block's first scores ahead of the
                    # output projection: their exps finish during C's pure-PE
                    # stretch, killing the bubble at the B(qb+1) boundary
                    if qb + 1 < NQB:
                        peeled[(qb + 1, 0)] = [emit_ps(qb + 1, 0, kt)
                                               for kt in range(3)]
                    # output projection for this query block
                    for qt in range(4 * qb, 4 * qb + 4):
                        for ddb in range(NQB):
                            py = ps_y.tile([128, 512], F32, tag="ps_y")
                            for ft in range(G):
                                nc.tensor.matmul(py[:], attn_sb[:, ft, ts(qt, 128)],
                                                 wo_sb[:, ft, ts(ddb, 512)],
                                                 start=(ft == 0), stop=(ft == G - 1))
                            y_sb = ystage.tile([128, 512], BF16, tag="ysb")
                            if ddb % 2 == 0:
                                nc.scalar.copy(y_sb[:], py[:])
                                nc.sync.dma_start(
                                    y.ap()[ts(qt, 128), ts(ddb, 512)], y_sb[:])
                            else:
                                nc.vector.tensor_copy(y_sb[:], py[:])
                                nc.gpsimd.dma_start(
                                    y.ap()[ts(qt, 128), ts(ddb, 512)], y_sb[:])

    nc.compile()
    _cache[key] = nc
    return nc


def _in_maps(hidden_q, Wq, Wk, Wv, Wo):
    bf = ml_dtypes.bfloat16

    def warr(wT):
        # [D, F] -> [128, NT*F] with arr[p, kt*F+f] = wT[kt*128+p, f]
        return np.ascontiguousarray(
            wT.reshape(NT, 128, F).transpose(1, 0, 2).reshape(128, NT * F)
        ).astype(bf)

    xs = (hidden_q.astype(np.float64) / math.sqrt(D))
    xTl = []
    for b in range(B):
        xT_b = xs[b].T                                       # [D, S]
        # [128, tb, kt*128] with arr[p, tb, kt*128+t] = xT[kt*128+p, tb*128+t]
        xTl.append(np.ascontiguousarray(
            xT_b.reshape(NT, 128, NT, 128).transpose(1, 2, 0, 3)
            .reshape(128, NT, NT * 128)).astype(bf))
    cos_t, sin_t = _rope_tables()
    cos_t, sin_t = cos_t.astype(bf), sin_t.astype(bf)
    masks = _mask_tiles().astype(bf)
    wo_s = Wo.astype(np.float64) / math.sqrt(H * DH)
    in_maps = []
    for c in range(8):
        b, g = c // G, c % G
        rows = slice(F * g, F * (g + 1))
        in_maps.append({
            "xT": xTl[b],
            "wqT": warr(Wq[rows, :].T),
            "wkT": warr(Wk[rows, :].T),
            "wvT": warr(Wv[rows, :].T),
            "woT": np.ascontiguousarray(wo_s[:, rows].T).astype(bf),
            "cos": cos_t, "sin": sin_t, "masks": masks,
        })
    return in_maps


def kernel(hidden_q, attention_mask, position_bias, Wq, Wk, Wv, Wo):
    hidden_q = np.asarray(hidden_q)
    Wq, Wk, Wv, Wo = (np.asarray(w) for w in (Wq, Wk, Wv, Wo))
    assert hidden_q.shape == (B, S, D)
    in_maps = _in_maps(hidden_q, Wq, Wk, Wv, Wo)
    nc = _build()
    res = run_bass_kernel_spmd(nc, in_maps, core_ids=list(range(8)))
    _cache["last_results"] = res
    out = np.zeros((B, S, D), np.float32)
    for c in range(8):
        out[c // G] += res.results[c]["y"].astype(np.float32)
    return out
